# revision 20
# baseline (speedup 1.0000x reference)
"""Trainium2 Bass kernel for nn_DiffMPC2 (100-step diagonal-QP SGD recursion).

The reference iterates  u <- u - LR*(2*q*u + p)  100 times, i.e. the affine
per-element map  u <- a*u + b  with  a = 1 - 0.02*q,  b = -0.01*p.  Closed
form:  u_100 = a^100 * u0 + b * S_100,  S_100 = sum_{k<100} a^k.

On-chip (per element, f32):
    L  = Ln(1 - 0.02*q)            ACT (free affine on input)
    P  = Exp(100*L)    = a^100     ACT
    R  = Exp(-Ln(q))   = 1/q       ACT x2
    E  = 0.5*P - 0.5               ACT (Identity w/ scale+bias)
    Sl = E*R           = -0.01*S   (LUT branch, exact for q not small)
    St = -1 + 0.99*q - 0.6468*q^2  (2nd-order Taylor of -0.01*S; exact for
                                    small q where (P-1) cancels badly)
    S' = q < QHI ? St : Sl         (copy_predicated)
    u  = P*u0 + S'*p

Sharding: pure data parallel, batch split across 8 cores.  Each core gets
131072 rows x 4 ctrl cols = 524288 elems laid out as [128, 4096] f32.
Only Q[:,12:], p[:,12:], u_init are touched (x_init is dead): 8 MB of HBM
traffic per core, which is the memory roofline for this problem.  The three
inputs are host-packed into one DRAM tensor ([q | p | u0] per partition)
so each chunk needs a single input DMA.

Written in raw bass (explicit per-engine programs + semaphores): the
container's walrus build only allows ONE sync-wait per compute instruction,
which the Tile scheduler's automatic sem assignment keeps exceeding.  With
raw bass every wait is its own instruction.  Double-buffered over
N_CHUNKS column chunks: DMA of chunk c+1 overlaps ACT/DVE of chunk c.
"""

import sys

for _p in (
    "/root/.axon_site",
    "/root/.axon_site/_ro/trn_rl_repo",
    "/root/.axon_site/_ro/pypackages",
):
    if _p not in sys.path:
        sys.path.append(_p)

import numpy as np

from concourse import bass, mybir
from concourse.bass_utils import run_bass_kernel_spmd

N_CORES = 8
B = 1048576
S_DIM = 12
C_DIM = 4
PARTS = 128
F_TOTAL = (B // N_CORES) * C_DIM // PARTS  # 4096
F_CHUNK = 1024
N_CHUNKS = F_TOTAL // F_CHUNK
NSLOT = 2  # double buffering

QHI = 0.04  # Taylor/LUT branch point

_nc_cache = None


def _build_bass():
    f32 = mybir.dt.float32
    u8 = mybir.dt.uint8
    Alu = mybir.AluOpType
    Act = mybir.ActivationFunctionType

    nc = bass.Bass()

    # Extra activation-bias constant (Bass only pre-registers 0/1).
    for val in (-0.5,):
        t = nc.alloc_sbuf_tensor(f"const-float32-{val}", [128, 1], f32)
        nc.gpsimd.memset(t.ap(), val)
        nc.const_aps.aps[(f32, val)] = t.ap()
    nc.all_engine_barrier()

    # Packed input: per partition [q | p | u0], each F_TOTAL wide.
    xin = nc.declare_dram_parameter("xin", [PARTS, 3 * F_TOTAL], f32, isOutput=False)
    uo = nc.declare_dram_parameter("uo", [PARTS, F_TOTAL], f32, isOutput=True)
    xr = xin.ap().rearrange("p (j f) -> p j f", j=3)

    def sb(name, cols, dtype=f32):
        return nc.alloc_sbuf_tensor(name, [PARTS, cols], dtype).ap()

    # Double-buffered tiles (cross-engine handoffs).
    tx = [sb(f"tx{s}", 3 * F_CHUNK).rearrange("p (j f) -> p j f", j=3) for s in range(NSLOT)]
    tP = [sb(f"tP{s}", F_CHUNK) for s in range(NSLOT)]
    tR = [sb(f"tR{s}", F_CHUNK) for s in range(NSLOT)]
    tE = [sb(f"tE{s}", F_CHUNK) for s in range(NSLOT)]
    # Engine-local scratch (in-order reuse is safe).
    tL = sb("tL", F_CHUNK)
    tG = sb("tG", F_CHUNK)
    tm = sb("tm", F_CHUNK, u8)
    th = sb("th", F_CHUNK)
    th2 = sb("th2", F_CHUNK)
    tSt = sb("tSt", F_CHUNK)
    tS = sb("tS", F_CHUNK)
    tr2 = sb("tr2", F_CHUNK)
    tr1 = sb("tr1", F_CHUNK)
    tout = sb("tout", F_TOTAL)

    with (
        nc.Block() as block,
        nc.semaphore("s_in") as s_in,
        nc.semaphore("s_act") as s_act,
        nc.semaphore("s_dve") as s_dve,
        nc.semaphore("s_out") as s_out,
    ):

        @block.sync
        def _(sp):
            for c in range(N_CHUNKS):
                if c >= NSLOT:
                    # tx slot reuse: both consumers of chunk c-NSLOT done.
                    sp.wait_ge(s_act, c - NSLOT + 1)
                    sp.wait_ge(s_dve, c - NSLOT + 1)
                sp.dma_start(
                    out=tx[c % NSLOT],
                    in_=xr[:, :, c * F_CHUNK : (c + 1) * F_CHUNK],
                ).then_inc(s_in, 16)
            for c in range(N_CHUNKS):
                sp.wait_ge(s_dve, c + 1)
                sp.dma_start(
                    out=uo.ap()[:, c * F_CHUNK : (c + 1) * F_CHUNK],
                    in_=tout[:, c * F_CHUNK : (c + 1) * F_CHUNK],
                ).then_inc(s_out, 16)
            sp.wait_ge(s_out, 16 * N_CHUNKS)

        @block.scalar
        def _(act):
            for c in range(N_CHUNKS):
                s = c % NSLOT
                tq = tx[s][:, 0, :]
                act.wait_ge(s_in, 16 * (c + 1))
                if c >= NSLOT:
                    # tP/tR/tE slot reuse: DVE chunk c-NSLOT must be done.
                    act.wait_ge(s_dve, c - NSLOT + 1)
                act.activation(tL, tq, Act.Ln, bias=1.0, scale=-0.02)
                act.activation(tP[s], tL, Act.Exp, bias=0.0, scale=100.0)
                act.activation(tG, tq, Act.Ln)
                act.activation(tR[s], tG, Act.Exp, bias=0.0, scale=-1.0)
                act.activation(tE[s], tP[s], Act.Identity, bias=-0.5, scale=0.5).then_inc(
                    s_act, 1
                )

        @block.vector
        def _(v):
            for c in range(N_CHUNKS):
                s = c % NSLOT
                tq = tx[s][:, 0, :]
                tp_ = tx[s][:, 1, :]
                tu = tx[s][:, 2, :]
                sl = slice(c * F_CHUNK, (c + 1) * F_CHUNK)
                v.wait_ge(s_in, 16 * (c + 1))
                v.tensor_scalar(tm, tq, QHI, None, Alu.is_lt)
                v.tensor_scalar(th, tq, -0.6468, 0.99, Alu.mult, Alu.add)
                v.tensor_tensor(th2, th, tq, Alu.mult)
                v.tensor_scalar_add(tSt, th2, -1.0)
                v.wait_ge(s_act, c + 1)
                v.tensor_mul(tS, tE[s], tR[s])
                v.copy_predicated(tS, tm, tSt)
                v.tensor_mul(tr2, tS, tp_)
                v.tensor_mul(tr1, tP[s], tu)
                v.tensor_add(tout[:, sl], tr1, tr2).then_inc(s_dve, 1)

    return nc


def _get_nc():
    global _nc_cache
    if _nc_cache is None:
        _nc_cache = _build_bass()
    return _nc_cache


def _prep_in_maps(Q, p, u_init):
    q_u = np.ascontiguousarray(Q[:, S_DIM:], dtype=np.float32).reshape(
        N_CORES, PARTS, F_TOTAL
    )
    p_u = np.ascontiguousarray(p[:, S_DIM:], dtype=np.float32).reshape(
        N_CORES, PARTS, F_TOTAL
    )
    u0 = np.ascontiguousarray(u_init, dtype=np.float32).reshape(
        N_CORES, PARTS, F_TOTAL
    )
    xin = np.concatenate([q_u, p_u, u0], axis=2)  # [8, 128, 3*F_TOTAL]
    return [{"xin": xin[c]} for c in range(N_CORES)]


def kernel(x_init, Q, p, u_init):
    assert Q.shape == (B, S_DIM + C_DIM) and u_init.shape == (B, C_DIM)
    nc = _get_nc()
    in_maps = _prep_in_maps(Q, p, u_init)
    res = run_bass_kernel_spmd(nc, in_maps, list(range(N_CORES)))
    out = np.stack([res.results[c]["uo"] for c in range(N_CORES)])
    return out.reshape(B, C_DIM)
